# revision 23
# baseline (speedup 1.0000x reference)
"""HGRN2Block kernel for 8 TRN2 NeuronCores.

Live path of the reference (the recurrence is dead code):
    x_proj = x @ W_proj + b_proj            # [B,L,3D]
    gate, _, ogate = split(x_proj, 3)       # middle third is DEAD
    out = gate * sigmoid(ogate)             # [B,L,D]
    out = out @ W_out + b_out               # [B,L,D]

Strategy:
  - Data-parallel over B*L rows: 16384 rows -> 2048 rows/core, no collectives.
  - Feature-major on device; host packs every tensor p-major so each weight
    matrix is ONE SBUF tile filled by a few large fully-contiguous DMAs
    (8-16 KB per partition line). Small/strided DMAs measured 60-70 GB/s vs
    ~190+ GB/s for large contiguous ones; SWDGE costs ~1.8 us of descriptor
    emission per dma_start regardless of size, so fewer+bigger wins.
  - Queue plan: sync HWDGE ring carries x (rb0 split fine for early start)
    then paired y-out tiles; scalar HWDGE ring carries biases + the fp8
    o-weights (needed first); gpsimd SWDGE carries the two big bf16 weight
    matrices as 1 MB transfers.
  - The output-gate projection runs in fp8 e4m3 with DoubleRow (2 k-slices
    per matmul, 4 matmuls per group instead of 8): its quantization error is
    damped by sigmoid' (~0.21 RMS), giving rel_err ~1.5e-2 (host-simulated,
    matches HW) vs the 2e-2 budget. DR matmuls issue at the same ~216 ns as
    bf16 (byte-limited moving stream) so the o-projection costs half.
  - o-proj weights pre-scaled x16 into e4m3's normal range; the sigmoid
    descales via its scale operand: sigmoid(psum/16 + bias).
  - Per row-block: all 8 fp8 o-groups first (small fp8 data arrives first),
    then 8 bf16 h-groups, then 8 bf16 layer-2 groups. Output DMA'd as bf16.
"""

import os

import numpy as np
import ml_dtypes

try:
    import concourse.bass as bass
except ImportError:
    import sys

    sys.path.insert(0, "/opt/trn_rl_repo")
    import concourse.bass as bass

import concourse.mybir as mybir
from concourse import bacc
from concourse.tile import TileContext
from concourse.bass_utils import run_bass_kernel_spmd

BF16 = ml_dtypes.bfloat16
F8E4 = ml_dtypes.float8_e4m3  # TRN fp8_e4m3 variant (max +-240)

B, L, D = 4, 4096, 1024
NCORES = 8
ROWS = B * L            # 16384
RPC = ROWS // NCORES    # 2048 rows per core
RB = 512                # moving free-dim per matmul (= one fp32 PSUM bank)
NRB = RPC // RB         # 4 row blocks per core
P = 128                 # SBUF partitions
KT = D // P             # 8 contraction tiles
WSCALE = 16.0           # o-proj weight pre-scale (descaled in the sigmoid)

_NC = None
LAST_RESULT = None      # BassKernelResults of the most recent run (for test.py)


def _build():
    nc = bacc.Bacc(trn_type="TRN2")
    f32 = mybir.dt.float32
    bf16 = mybir.dt.bfloat16
    f8 = mybir.dt.float8e4
    DR = mybir.MatmulPerfMode.DoubleRow

    # Weights packed p-major: dim1 flat index = m*8+k (wo8/wg) or n*8+m (wu).
    x8 = nc.dram_tensor("x8", [NRB, P, KT, RB], f8, kind="ExternalInput")
    xb = nc.dram_tensor("xb", [NRB, P, KT, RB], bf16, kind="ExternalInput")
    wo8 = nc.dram_tensor("wo8", [P, KT * KT, P], f8, kind="ExternalInput")
    wg = nc.dram_tensor("wg", [P, KT * KT, P], bf16, kind="ExternalInput")
    wu = nc.dram_tensor("wu", [P, KT * KT, P], bf16, kind="ExternalInput")
    # all three biases in one [128, 24] tensor: columns [bg | bo | bu]
    bb = nc.dram_tensor("bb", [P, 3 * KT], f32, kind="ExternalInput")
    # y row-block-wide: [rb, p, n*512+cb] = out[rb*512+cb, n*128+p]
    y = nc.dram_tensor("y", [NRB, P, KT * RB], bf16, kind="ExternalOutput")

    with TileContext(nc) as tc:
        with (
            tc.tile_pool(name="const", bufs=1) as cpool,
            tc.tile_pool(name="work", bufs=2) as wpool,
            tc.tile_pool(name="outp", bufs=2) as opool,
            tc.tile_pool(name="ps", bufs=2, space="PSUM") as pspool,
        ):
            bbS = cpool.tile([P, 3 * KT], f32, tag="bb", name="bbS")
            bgS = bbS[:, 0:KT]
            boS = bbS[:, KT:2 * KT]
            buS = bbS[:, 2 * KT:3 * KT]

            # Warm-up: HAM starts the PE clock-gated at 1.2 GHz and ungates
            # after ~3.4us of sustained activity. A few spins on a zeroed tile
            # (no DMA deps) start the warm-up clock during the DMA prologue.
            wz = cpool.tile([P, RB], bf16, tag="wz", name="wz")
            nc.vector.memset(wz, 0.0)
            # 12 spins (~5.1us cold) bridge until the first real matmul
            # (~12.9us, the x8+wo8 DMA floor) with no >3.4us PE idle window,
            # so HAM is ungated before real work. Spins share the "po" PSUM
            # tag (no readers, freed at stop) to save a bank.
            for i in range(12):
                spin = pspool.tile([P, RB], f32, tag="po", name=f"spin{i}", bufs=4)
                nc.tensor.matmul(spin, lhsT=wz[:, :P], rhs=wz, start=True, stop=True)

            # One SBUF tile per packed tensor.
            x8S = [cpool.tile([P, KT, RB], f8, tag=f"x8_{r}", name=f"x8S{r}")
                   for r in range(NRB)]
            xbS = [cpool.tile([P, KT, RB], bf16, tag=f"xb_{r}", name=f"xbS{r}")
                   for r in range(NRB)]
            wo8S = cpool.tile([P, KT * KT, P], f8, tag="wo8", name="wo8S")
            wgS = cpool.tile([P, KT * KT, P], bf16, tag="wg", name="wgS")
            wuS = cpool.tile([P, KT * KT, P], bf16, tag="wu", name="wuS")

            # The 16 SDMA engines are ONE shared pool (~200ns per descriptor
            # service, one descriptor per partition per DMA); concurrent busy
            # rings split it per-packet. So: a SINGLE ring carrying every
            # input in exact need order owns the whole pool (~350-430 GB/s
            # with >=4KB lines), and the ring order IS the priority order.
            W2 = KT * KT // 2
            W4 = KT * KT // 4
            nc.sync.dma_start(out=x8S[0], in_=x8[0, :, :, :])
            nc.sync.dma_start(out=wo8S[:, 0:W2, :], in_=wo8[:, 0:W2, :])
            nc.sync.dma_start(out=wo8S[:, W2:2 * W2, :], in_=wo8[:, W2:2 * W2, :])
            nc.sync.dma_start(out=xbS[0], in_=xb[0, :, :, :])
            for q in range(4):  # gate weights in quarters, paced vs h-phase
                nc.sync.dma_start(out=wgS[:, q * W4:(q + 1) * W4, :],
                                  in_=wg[:, q * W4:(q + 1) * W4, :])
            for h in range(2):
                nc.sync.dma_start(out=wuS[:, h * W2:(h + 1) * W2, :],
                                  in_=wu[:, h * W2:(h + 1) * W2, :])
            for r in range(1, NRB):
                nc.sync.dma_start(out=x8S[r], in_=x8[r, :, :, :])
                nc.sync.dma_start(out=xbS[r], in_=xb[r, :, :, :])

            # scalar HWDGE ring: only the small bias tile (needed by the
            # first sigmoid ~10.5us), tiny enough not to disturb the pool.
            nc.scalar.dma_start(out=bbS, in_=bb[:, :])

            for rb in range(NRB):
                # ---- o-phase: fp8 DoubleRow, 4 matmuls per group ----
                sigs = []
                for m in range(KT):
                    po = pspool.tile([P, RB], f32, tag="po", name=f"po{rb}_{m}",
                                     bufs=4)
                    for j in range(KT // 2):
                        s = m * KT + 2 * j
                        nc.tensor.matmul(
                            po,
                            lhsT=wo8S[:, s:s + 2, :],
                            rhs=x8S[rb][:, 2 * j:2 * j + 2, :],
                            start=(j == 0), stop=(j == KT // 2 - 1),
                            perf_mode=DR,
                        )
                    sig = opool.tile([P, RB], bf16, tag=f"sig{m}",
                                     name=f"sig{rb}_{m}")
                    nc.scalar.activation(
                        out=sig, in_=po,
                        func=mybir.ActivationFunctionType.Sigmoid,
                        bias=boS[:, m:m + 1], scale=1.0 / WSCALE,
                    )
                    sigs.append(sig)
                # ---- h-phase: bf16 gate proj; g = (h + bg) * sig ----
                gS = []
                for m in range(KT):
                    ph = pspool.tile([P, RB], f32, tag="ph", name=f"ph{rb}_{m}")
                    for k in range(KT):
                        s = m * KT + k
                        nc.tensor.matmul(
                            ph, lhsT=wgS[:, s:s + 1, :],
                            rhs=xbS[rb][:, k:k + 1, :],
                            start=(k == 0), stop=(k == KT - 1),
                        )
                    g = wpool.tile([P, RB], bf16, tag=f"g{m}", name=f"g{rb}_{m}")
                    nc.vector.scalar_tensor_tensor(
                        out=g, in0=ph, scalar=bgS[:, m:m + 1], in1=sigs[m],
                        op0=mybir.AluOpType.add, op1=mybir.AluOpType.mult,
                    )
                    gS.append(g)
                # ---- layer 2: y = g @ W_out (+ b_out), bf16, rb-wide out ----
                yo = opool.tile([P, KT * RB], bf16, tag="yo",
                                name=f"yo{rb}", bufs=3)
                for n in range(KT):
                    py = pspool.tile([P, RB], f32, tag="py", name=f"py{rb}_{n}")
                    for m in range(KT):
                        s = n * KT + m
                        nc.tensor.matmul(
                            py, lhsT=wuS[:, s:s + 1, :], rhs=gS[m],
                            start=(m == 0), stop=(m == KT - 1),
                        )
                    nc.vector.tensor_scalar_add(
                        yo[:, n * RB:(n + 1) * RB], py, buS[:, n:n + 1])
                    # issue each half as soon as its four slices are done so
                    # the final DMA's descriptors overlap compute
                    if n == KT // 2 - 1:
                        nc.sync.dma_start(out=y[rb, :, 0:KT * RB // 2],
                                          in_=yo[:, 0:KT * RB // 2])
                    elif n == KT - 1:
                        nc.sync.dma_start(out=y[rb, :, KT * RB // 2:KT * RB],
                                          in_=yo[:, KT * RB // 2:KT * RB])
    nc.finalize()
    return nc


def kernel(x, W_proj, b_proj, W_out, b_out, layer_idx=0, num_layers=12):
    global _NC, LAST_RESULT
    x = np.asarray(x, dtype=np.float32)
    W_proj = np.asarray(W_proj, dtype=np.float32)
    b_proj = np.asarray(b_proj, dtype=np.float32)
    W_out = np.asarray(W_out, dtype=np.float32)
    b_out = np.asarray(b_out, dtype=np.float32)

    Wg = W_proj[:, :D]
    Wo = W_proj[:, 2 * D:3 * D]

    def pack_w(w):
        # [D, D] -> [p, a*8+b, c] with out[p, a*8+b, c] = w[b*128+p, a*128+c]
        return np.ascontiguousarray(
            w.reshape(KT, P, KT, P).transpose(1, 2, 0, 3).reshape(P, KT * KT, P)
        )

    wgp = pack_w(Wg).astype(BF16)
    wo8p = pack_w(Wo * WSCALE).astype(F8E4)
    wup = pack_w(W_out).astype(BF16)
    bbp = np.ascontiguousarray(np.concatenate([
        b_proj[:D].reshape(KT, P).T,
        b_proj[2 * D:3 * D].reshape(KT, P).T,
        b_out.reshape(KT, P).T,
    ], axis=1))

    xf = x.reshape(ROWS, D)
    in_maps = []
    for c in range(NCORES):
        # [rb, p, k, cb] with xc[rb, p, k, cb] = x_core[rb*512+cb, k*128+p]
        xc = np.ascontiguousarray(
            xf[c * RPC:(c + 1) * RPC].reshape(NRB, RB, KT, P).transpose(0, 3, 2, 1)
        )
        in_maps.append({
            "x8": xc.astype(F8E4), "xb": xc.astype(BF16),
            "wo8": wo8p, "wg": wgp, "wu": wup, "bb": bbp,
        })

    if _NC is None:
        _NC = _build()

    trace = os.environ.get("HGRN_TRACE", "0") == "1"
    LAST_RESULT = run_bass_kernel_spmd(
        _NC, in_maps, core_ids=list(range(NCORES)), trace=trace,
        tmpdir=os.environ.get("HGRN_TMPDIR"),
    )
    yout = np.empty((ROWS, D), dtype=np.float32)
    for c in range(NCORES):
        yc = np.asarray(LAST_RESULT.results[c]["y"])  # [rb, p, 4096] bf16
        yc = yc.reshape(NRB, P, KT, RB)               # [rb, p, n, cb]
        yout[c * RPC:(c + 1) * RPC] = (
            yc.transpose(0, 3, 2, 1).reshape(RPC, D).astype(np.float32)
        )
    return yout.reshape(B, L, D)


# revision 24
# speedup vs baseline: 1.0088x; 1.0088x over previous
"""HGRN2Block kernel for 8 TRN2 NeuronCores.

Live path of the reference (the recurrence is dead code):
    x_proj = x @ W_proj + b_proj            # [B,L,3D]
    gate, _, ogate = split(x_proj, 3)       # middle third is DEAD
    out = gate * sigmoid(ogate)             # [B,L,D]
    out = out @ W_out + b_out               # [B,L,D]

Strategy:
  - Data-parallel over B*L rows: 16384 rows -> 2048 rows/core, no collectives.
  - Feature-major on device; host packs every tensor p-major so each weight
    matrix is ONE SBUF tile filled by a few large fully-contiguous DMAs.
    Measured on HW: the 16-engine SDMA pool services ~13-15 ns per
    descriptor, one descriptor per partition per DMA, and concurrent rings
    split the pool per-packet. So all inputs ride ONE ring (sync HWDGE) in
    exact need order with maximal per-partition lines (4-16 KB); only the
    tiny bias tile uses the scalar ring. y-outs follow on the sync ring.
  - The output-gate projection runs in fp8 e4m3 with DoubleRow (2 k-slices
    per matmul, 4 matmuls per group instead of 8): its quantization error is
    damped by sigmoid' (~0.21 RMS), giving rel_err ~1.5e-2 (host-simulated,
    matches HW) vs the 2e-2 budget. DR matmuls issue at the same ~216 ns as
    bf16 (byte-limited moving stream) so the o-projection costs half.
  - o-proj weights pre-scaled x16 into e4m3's normal range; the sigmoid
    descales via its scale operand: sigmoid(psum/16 + bias).
  - Per row-block: all 8 fp8 o-groups first (small fp8 data arrives first),
    then 8 bf16 h-groups, then 8 bf16 layer-2 groups. Output DMA'd as bf16.
"""

import os

import numpy as np
import ml_dtypes

try:
    import concourse.bass as bass
except ImportError:
    import sys

    sys.path.insert(0, "/opt/trn_rl_repo")
    import concourse.bass as bass

import concourse.mybir as mybir
from concourse import bacc
from concourse.tile import TileContext
from concourse.bass_utils import run_bass_kernel_spmd

BF16 = ml_dtypes.bfloat16
F8E4 = ml_dtypes.float8_e4m3  # TRN fp8_e4m3 variant (max +-240)

B, L, D = 4, 4096, 1024
NCORES = 8
ROWS = B * L            # 16384
RPC = ROWS // NCORES    # 2048 rows per core
RB = 512                # moving free-dim per matmul (= one fp32 PSUM bank)
NRB = RPC // RB         # 4 row blocks per core
P = 128                 # SBUF partitions
KT = D // P             # 8 contraction tiles
WSCALE = 16.0           # o-proj weight pre-scale (descaled in the sigmoid)

_NC = None
LAST_RESULT = None      # BassKernelResults of the most recent run (for test.py)


def _build():
    nc = bacc.Bacc(trn_type="TRN2")
    f32 = mybir.dt.float32
    bf16 = mybir.dt.bfloat16
    f8 = mybir.dt.float8e4
    DR = mybir.MatmulPerfMode.DoubleRow

    # Weights packed p-major: dim1 flat index = m*8+k (wo8/wg) or n*8+m (wu).
    x8 = nc.dram_tensor("x8", [NRB, P, KT, RB], f8, kind="ExternalInput")
    xb = nc.dram_tensor("xb", [NRB, P, KT, RB], bf16, kind="ExternalInput")
    wo8 = nc.dram_tensor("wo8", [P, KT * KT, P], f8, kind="ExternalInput")
    wg = nc.dram_tensor("wg", [P, KT * KT, P], bf16, kind="ExternalInput")
    wu = nc.dram_tensor("wu", [P, KT * KT, P], bf16, kind="ExternalInput")
    # all three biases in one [128, 24] tensor: columns [bg | bo | bu]
    bb = nc.dram_tensor("bb", [P, 3 * KT], f32, kind="ExternalInput")
    # y row-block-wide: [rb, p, n*512+cb] = out[rb*512+cb, n*128+p]
    y = nc.dram_tensor("y", [NRB, P, KT * RB], bf16, kind="ExternalOutput")

    with TileContext(nc) as tc:
        with (
            tc.tile_pool(name="const", bufs=1) as cpool,
            tc.tile_pool(name="work", bufs=2) as wpool,
            tc.tile_pool(name="outp", bufs=2) as opool,
            tc.tile_pool(name="ps", bufs=2, space="PSUM") as pspool,
        ):
            bbS = cpool.tile([P, 3 * KT], f32, tag="bb", name="bbS")
            bgS = bbS[:, 0:KT]
            boS = bbS[:, KT:2 * KT]
            buS = bbS[:, 2 * KT:3 * KT]

            # Warm-up: HAM starts the PE clock-gated at 1.2 GHz and ungates
            # after ~3.4us of sustained activity. A few spins on a zeroed tile
            # (no DMA deps) start the warm-up clock during the DMA prologue.
            wz = cpool.tile([P, RB], bf16, tag="wz", name="wz")
            nc.vector.memset(wz, 0.0)
            # 12 spins (~5.1us cold) bridge until the first real matmul
            # (~12.9us, the x8+wo8 DMA floor) with no >3.4us PE idle window,
            # so HAM is ungated before real work. Spins share the "po" PSUM
            # tag (no readers, freed at stop) to save a bank.
            for i in range(12):
                spin = pspool.tile([P, RB], f32, tag="po", name=f"spin{i}", bufs=4)
                nc.tensor.matmul(spin, lhsT=wz[:, :P], rhs=wz, start=True, stop=True)

            # One SBUF tile per packed tensor.
            x8S = [cpool.tile([P, KT, RB], f8, tag=f"x8_{r}", name=f"x8S{r}")
                   for r in range(NRB)]
            xbS = [cpool.tile([P, KT, RB], bf16, tag=f"xb_{r}", name=f"xbS{r}")
                   for r in range(NRB)]
            wo8S = cpool.tile([P, KT * KT, P], f8, tag="wo8", name="wo8S")
            wgS = cpool.tile([P, KT * KT, P], bf16, tag="wg", name="wgS")
            wuS = cpool.tile([P, KT * KT, P], bf16, tag="wu", name="wuS")

            # The 16 SDMA engines are ONE shared pool (~200ns per descriptor
            # service, one descriptor per partition per DMA); concurrent busy
            # rings split it per-packet. So: a SINGLE ring carrying every
            # input in exact need order owns the whole pool (~350-430 GB/s
            # with >=4KB lines), and the ring order IS the priority order.
            W2 = KT * KT // 2
            W4 = KT * KT // 4
            nc.sync.dma_start(out=x8S[0], in_=x8[0, :, :, :])
            nc.sync.dma_start(out=wo8S[:, 0:W2, :], in_=wo8[:, 0:W2, :])
            nc.sync.dma_start(out=wo8S[:, W2:2 * W2, :], in_=wo8[:, W2:2 * W2, :])
            nc.sync.dma_start(out=xbS[0], in_=xb[0, :, :, :])
            for q in range(4):  # gate weights in quarters, paced vs h-phase
                nc.sync.dma_start(out=wgS[:, q * W4:(q + 1) * W4, :],
                                  in_=wg[:, q * W4:(q + 1) * W4, :])
            for h in range(2):
                nc.sync.dma_start(out=wuS[:, h * W2:(h + 1) * W2, :],
                                  in_=wu[:, h * W2:(h + 1) * W2, :])
            for r in range(1, NRB):
                nc.sync.dma_start(out=x8S[r], in_=x8[r, :, :, :])
                nc.sync.dma_start(out=xbS[r], in_=xb[r, :, :, :])

            # scalar HWDGE ring: only the small bias tile (needed by the
            # first sigmoid ~10.5us), tiny enough not to disturb the pool.
            nc.scalar.dma_start(out=bbS, in_=bb[:, :])

            for rb in range(NRB):
                # ---- o-phase: fp8 DoubleRow, 4 matmuls per group ----
                sigs = []
                for m in range(KT):
                    po = pspool.tile([P, RB], f32, tag="po", name=f"po{rb}_{m}",
                                     bufs=4)
                    for j in range(KT // 2):
                        s = m * KT + 2 * j
                        nc.tensor.matmul(
                            po,
                            lhsT=wo8S[:, s:s + 2, :],
                            rhs=x8S[rb][:, 2 * j:2 * j + 2, :],
                            start=(j == 0), stop=(j == KT // 2 - 1),
                            perf_mode=DR,
                        )
                    sig = opool.tile([P, RB], bf16, tag=f"sig{m}",
                                     name=f"sig{rb}_{m}")
                    nc.scalar.activation(
                        out=sig, in_=po,
                        func=mybir.ActivationFunctionType.Sigmoid,
                        bias=boS[:, m:m + 1], scale=1.0 / WSCALE,
                    )
                    sigs.append(sig)
                # ---- h-phase: bf16 gate proj; g = (h + bg) * sig ----
                gS = []
                for m in range(KT):
                    ph = pspool.tile([P, RB], f32, tag="ph", name=f"ph{rb}_{m}")
                    for k in range(KT):
                        s = m * KT + k
                        nc.tensor.matmul(
                            ph, lhsT=wgS[:, s:s + 1, :],
                            rhs=xbS[rb][:, k:k + 1, :],
                            start=(k == 0), stop=(k == KT - 1),
                        )
                    g = wpool.tile([P, RB], bf16, tag=f"g{m}", name=f"g{rb}_{m}")
                    nc.vector.scalar_tensor_tensor(
                        out=g, in0=ph, scalar=bgS[:, m:m + 1], in1=sigs[m],
                        op0=mybir.AluOpType.add, op1=mybir.AluOpType.mult,
                    )
                    gS.append(g)
                # ---- layer 2: y = g @ W_out (+ b_out), bf16, rb-wide out ----
                yo = opool.tile([P, KT * RB], bf16, tag="yo",
                                name=f"yo{rb}", bufs=3)
                for n in range(KT):
                    py = pspool.tile([P, RB], f32, tag="py", name=f"py{rb}_{n}")
                    for m in range(KT):
                        s = n * KT + m
                        nc.tensor.matmul(
                            py, lhsT=wuS[:, s:s + 1, :], rhs=gS[m],
                            start=(m == 0), stop=(m == KT - 1),
                        )
                    nc.vector.tensor_scalar_add(
                        yo[:, n * RB:(n + 1) * RB], py, buS[:, n:n + 1])
                    # issue each half as soon as its four slices are done so
                    # the final DMA's descriptors overlap compute
                    if n == KT // 2 - 1:
                        nc.sync.dma_start(out=y[rb, :, 0:KT * RB // 2],
                                          in_=yo[:, 0:KT * RB // 2])
                    elif n == KT - 1:
                        nc.sync.dma_start(out=y[rb, :, KT * RB // 2:KT * RB],
                                          in_=yo[:, KT * RB // 2:KT * RB])
    nc.finalize()
    return nc


def kernel(x, W_proj, b_proj, W_out, b_out, layer_idx=0, num_layers=12):
    global _NC, LAST_RESULT
    x = np.asarray(x, dtype=np.float32)
    W_proj = np.asarray(W_proj, dtype=np.float32)
    b_proj = np.asarray(b_proj, dtype=np.float32)
    W_out = np.asarray(W_out, dtype=np.float32)
    b_out = np.asarray(b_out, dtype=np.float32)

    Wg = W_proj[:, :D]
    Wo = W_proj[:, 2 * D:3 * D]

    def pack_w(w):
        # [D, D] -> [p, a*8+b, c] with out[p, a*8+b, c] = w[b*128+p, a*128+c]
        return np.ascontiguousarray(
            w.reshape(KT, P, KT, P).transpose(1, 2, 0, 3).reshape(P, KT * KT, P)
        )

    wgp = pack_w(Wg).astype(BF16)
    wo8p = pack_w(Wo * WSCALE).astype(F8E4)
    wup = pack_w(W_out).astype(BF16)
    bbp = np.ascontiguousarray(np.concatenate([
        b_proj[:D].reshape(KT, P).T,
        b_proj[2 * D:3 * D].reshape(KT, P).T,
        b_out.reshape(KT, P).T,
    ], axis=1))

    xf = x.reshape(ROWS, D)
    in_maps = []
    for c in range(NCORES):
        # [rb, p, k, cb] with xc[rb, p, k, cb] = x_core[rb*512+cb, k*128+p]
        xc = np.ascontiguousarray(
            xf[c * RPC:(c + 1) * RPC].reshape(NRB, RB, KT, P).transpose(0, 3, 2, 1)
        )
        in_maps.append({
            "x8": xc.astype(F8E4), "xb": xc.astype(BF16),
            "wo8": wo8p, "wg": wgp, "wu": wup, "bb": bbp,
        })

    if _NC is None:
        _NC = _build()

    trace = os.environ.get("HGRN_TRACE", "0") == "1"
    LAST_RESULT = run_bass_kernel_spmd(
        _NC, in_maps, core_ids=list(range(NCORES)), trace=trace,
        tmpdir=os.environ.get("HGRN_TMPDIR"),
    )
    yout = np.empty((ROWS, D), dtype=np.float32)
    for c in range(NCORES):
        yc = np.asarray(LAST_RESULT.results[c]["y"])  # [rb, p, 4096] bf16
        yc = yc.reshape(NRB, P, KT, RB)               # [rb, p, n, cb]
        yout[c * RPC:(c + 1) * RPC] = (
            yc.transpose(0, 3, 2, 1).reshape(RPC, D).astype(np.float32)
        )
    return yout.reshape(B, L, D)
